# revision 64
# baseline (speedup 1.0000x reference)
"""Trainium2 Bass kernel for nn_Attention_78280073937702.

Dense transformer attention block (prefill, B=1, S=2048, H=4096, 32 heads,
head_dim=128, fp32) sharded tensor-parallel over heads across 8 NeuronCores
(4 heads per core).  Per core:

  1. transpose hidden -> h-major (PE transpose, cast bf16)
  2. QKV projection (bf16 matmuls, fp32 accum):
       Q^T,K^T produced head-dim-major [d, s]; V produced natural [s, d]
  3. RoPE applied to Q^T/K^T (signed half-swap via PE permutation matmul +
     DVE elementwise with host-provided cos/sin tables)
  4. causal attention per head in fp32r:
       S^T[k,q] tiles -> exp (ACT) -> triangular-mask diag blocks ->
       Z via DVE tree-sum + ones-matmul colsum-broadcast ->
       U^T = V^T E (PSUM accum) -> attn^T = U^T * (1/Z)  (bf16)
  5. AllGather attn^T (bf16) across the 8 cores
  6. o_proj: out^T[o-slice, s] = Wo_slice @ attn_full^T (bf16 matmuls)

Host side: shards W_pack/W_o by head, builds RoPE tables from position_ids,
concatenates the 8 per-core out^T slices and transposes to [1, S, H].
"""

import os
import sys
from contextlib import ExitStack

import numpy as np

for _p in ("/opt/trn_rl_repo", os.path.expanduser("~/.axon_site/_ro/trn_rl_repo")):
    if os.path.isdir(_p) and _p not in sys.path:
        sys.path.insert(0, _p)

import concourse.bacc as bacc  # noqa: E402
import concourse.bass as bass  # noqa: E402
import concourse.mybir as mybir  # noqa: E402
import concourse.tile as tile  # noqa: E402
from concourse.alu_op_type import AluOpType  # noqa: E402
from concourse.bass_utils import run_bass_kernel_spmd  # noqa: E402

F32 = mybir.dt.float32
F32R = mybir.dt.float32r
BF16 = mybir.dt.bfloat16
EXPF = mybir.ActivationFunctionType.Exp

N_CORES = 8
S = 2048
H = 4096
D = 128
N_HEADS = 32
NH_LOC = N_HEADS // N_CORES  # 4 heads per core
P = 128
HT = H // P  # 32 h-tiles
ST = S // P  # 16 s-tiles
SL = 512  # s-slice width for matmul free dim
NSL = S // SL  # 4
PQK = 2 * NH_LOC  # 8 p-tiles for q,k
VC = NH_LOC * D  # 512 local v columns
OC = H // N_CORES  # 512 output columns per core
NORM = 1.0 / float(np.sqrt(D))


def r32(ap):
    return ap.bitcast(F32R)


def _load_transposed(nc, dram_row_block, nat_pool, tps_pool, sink):
    """Load a [P, H] fp32 row-block of `dram_row_block`, cast to bf16 on DVE,
    and PE-transpose (bf16, 2x faster) into h-major tiles 4 at a time (one
    PSUM bank), delivering each [P, 4, P] bf16 PSUM tile to
    sink(ht_base, psum_tile)."""
    for half in range(2):
        nt = nat_pool.tile([P, H // 2], F32, tag="nat")
        nc.sync.dma_start(nt, dram_row_block[:, half * (H // 2):(half + 1) * (H // 2)])
        ntb = nat_pool.tile([P, H // 2], BF16, tag="natb", bufs=1)
        nc.vector.tensor_copy(ntb, nt)
        for grp in range(HT // 8):
            ps = tps_pool.tile([P, 4, P], BF16, tag="t")
            for u in range(4):
                hh = grp * 4 + u
                nc.tensor.transpose(
                    ps[:, u, :], ntb[:, hh * P:(hh + 1) * P], _load_transposed.identb)
            sink(half * (HT // 2) + grp * 4, ps)


def build_nc():
    nc = bacc.Bacc("TRN2", target_bir_lowering=False, num_devices=N_CORES)

    hid_d = nc.dram_tensor("hidden", [S, H], F32, kind="ExternalInput")
    wpk_d = nc.dram_tensor("w_pack", [PQK * P + VC, H], F32, kind="ExternalInput")
    wo_d = nc.dram_tensor("w_o", [OC, H], F32, kind="ExternalInput")
    cos_d = nc.dram_tensor("cos_t", [D, S], F32, kind="ExternalInput")
    sin_d = nc.dram_tensor("sin_t", [D, S], F32, kind="ExternalInput")
    out_d = nc.dram_tensor("out_t", [OC, S], F32, kind="ExternalOutput")

    with tile.TileContext(nc) as tc, ExitStack() as ctx:
        dram = ctx.enter_context(tc.tile_pool(name="dram", bufs=1, space="DRAM"))
        qkT_d = dram.tile([PQK, P, S], F32R)  # roped q^T,k^T per head
        v_d = dram.tile([S, VC], F32R)  # V natural, all local heads
        attn_loc = dram.tile([OC, S], BF16)
        # one gather buffer per local head index: rows = (core, d)
        attn_all = [
            dram.tile([N_CORES * D, S], BF16, addr_space="Shared",
                      name=f"attn_all{h}")
            for h in range(NH_LOC)
        ]

        # ---------------- constants ----------------
        # persistent consts kept through the kernel; the f32 scratch used
        # only to BUILD them lives in a pool that frees before the phases.
        consts = ctx.enter_context(tc.tile_pool(name="consts", bufs=1))
        identb = consts.tile([P, P], BF16)
        tri01 = consts.tile([P, P], F32)
        p_swap_r = consts.tile([P, P], F32R)
        ones_r = consts.tile([P, P], F32R)
        with ExitStack() as cb:
            ctmp = cb.enter_context(tc.tile_pool(name="ctmp", bufs=1))
            ones_t = ctmp.tile([P, P], F32)
            nc.gpsimd.memset(ones_t, 1.0)
            ident = ctmp.tile([P, P], F32)
            nc.gpsimd.memset(ident, 0.0)
            nc.gpsimd.affine_select(
                out=ident, in_=ones_t, compare_op=AluOpType.is_equal,
                fill=0.0, base=0, channel_multiplier=1, pattern=[[-1, P]],
            )
            nc.vector.tensor_copy(identb, ident)
            # signed half-swap as lhsT (so that lhsT.T @ x = rot_half(x)):
            # lhsT[i, i+64] = +1 (i < 64), lhsT[i, i-64] = -1 (i >= 64)
            neg_t = ctmp.tile([P, P], F32)
            nc.gpsimd.memset(neg_t, -1.0)
            sw_pos = ctmp.tile([P, P], F32)
            nc.gpsimd.affine_select(
                out=sw_pos, in_=ones_t, compare_op=AluOpType.is_equal,
                fill=0.0, base=-64, channel_multiplier=-1, pattern=[[1, P]],
            )  # iota = -64 - i + j == 0  ->  j = i + 64
            sw_neg = ctmp.tile([P, P], F32)
            nc.gpsimd.affine_select(
                out=sw_neg, in_=neg_t, compare_op=AluOpType.is_equal,
                fill=0.0, base=64, channel_multiplier=-1, pattern=[[1, P]],
            )  # iota = 64 - i + j == 0  ->  j = i - 64
            p_swap = ctmp.tile([P, P], F32)
            nc.vector.tensor_add(p_swap, sw_pos, sw_neg)
            # upper-triangular-with-diag 0/1 keep-mask in [k, q] layout:
            # keep where q - k >= 0
            nc.gpsimd.affine_select(
                out=tri01, in_=ones_t, compare_op=AluOpType.is_ge,
                fill=0.0, base=0, channel_multiplier=-1, pattern=[[1, P]],
            )
            # fp32r copies for fp32r matmuls (producer must round to f32r)
            nc.vector.tensor_copy(p_swap_r, p_swap)
            nc.vector.tensor_copy(ones_r, ones_t)
        _load_transposed.identb = identb

        with ExitStack() as ab:  # phases A, B1, B2: hidT + loaders live here
            nat = ab.enter_context(tc.tile_pool(name="nat", bufs=2))
            tps = ab.enter_context(tc.tile_pool(name="tpsum", bufs=2, space="PSUM"))
            hidT_pool = ab.enter_context(tc.tile_pool(name="hidT", bufs=1))
            hidT = hidT_pool.tile([P, HT, S], BF16)  # 128 KB/part

            # ---------- phases A+B1 interleaved: hidden^T + V projection ----
            # wtv is built first so the V matmuls can cover the PE while the
            # (DMA-bound) hidden loads stream in.
            with ExitStack() as b1:
                wtv_pool = b1.enter_context(tc.tile_pool(name="wtv", bufs=1))
                vps_pool = b1.enter_context(
                    tc.tile_pool(name="vpsum", bufs=2, space="PSUM"))
                vstage = b1.enter_context(tc.tile_pool(name="vstage", bufs=2))
                wtv = wtv_pool.tile([P, HT, VC], BF16)  # 32 KB/part
                for i in range(NH_LOC):
                    pt = PQK + i
                    def sink_w(ht, ps, i=i):
                        nc.scalar.copy(wtv[:, ht:ht + 4, i * P:(i + 1) * P], ps)
                    _load_transposed(
                        nc, wpk_d[pt * P:(pt + 1) * P, :], nat, tps, sink_w)
                for st in range(ST):
                    def sink_hid(ht, ps, st=st):
                        nc.scalar.copy(hidT[:, ht:ht + 4, st * P:(st + 1) * P], ps)
                    _load_transposed(
                        nc, hid_d[st * P:(st + 1) * P, :], nat, tps, sink_hid)
                    vps = vps_pool.tile([P, VC], F32, tag="v")
                    for ht in range(HT):
                        nc.tensor.matmul(
                            vps, hidT[:, ht, st * P:(st + 1) * P], wtv[:, ht, :],
                            start=(ht == 0), stop=(ht == HT - 1),
                        )
                    vsb = vstage.tile([P, VC], F32R, tag="vs")
                    nc.scalar.copy(vsb, vps)
                    nc.sync.dma_start(v_d[st * P:(st + 1) * P, :], vsb)

            # ---------------- phase B2: Q/K projection + RoPE ----------------
            with ExitStack() as b2:
                trig = b2.enter_context(tc.tile_pool(name="trig", bufs=1))
                cosT = trig.tile([D, S], F32)
                sinT = trig.tile([D, S], F32)
                nc.sync.dma_start(cosT, cos_d[:, :])
                nc.sync.dma_start(sinT, sin_d[:, :])

                wt_pool = b2.enter_context(tc.tile_pool(name="wt", bufs=2))
                qkps_pool = b2.enter_context(
                    tc.tile_pool(name="qkpsum", bufs=1, space="PSUM"))
                rps_pool = b2.enter_context(
                    tc.tile_pool(name="ropepsum", bufs=2, space="PSUM"))
                rstage = b2.enter_context(tc.tile_pool(name="rstage", bufs=2))

                for pt in range(PQK):
                    wt = wt_pool.tile([P, HT, P], BF16, tag="wt")
                    def sink_qk(ht, ps, wt=wt):
                        nc.scalar.copy(wt[:, ht:ht + 4, :], ps)
                    _load_transposed(
                        nc, wpk_d[pt * P:(pt + 1) * P, :], nat, tps, sink_qk)

                    qk_ps = [qkps_pool.tile([P, SL], F32, tag=f"qk{sl}",
                                            name=f"qkps{sl}")
                             for sl in range(NSL)]
                    for ht in range(HT):
                        for sl in range(NSL):
                            nc.tensor.matmul(
                                qk_ps[sl], wt[:, ht, :],
                                hidT[:, ht, sl * SL:(sl + 1) * SL],
                                start=(ht == 0), stop=(ht == HT - 1),
                            )
                    for sl in range(NSL):
                        qt = rstage.tile([P, SL], F32R, tag="qt")
                        nc.scalar.copy(qt, qk_ps[sl])
                        rps = rps_pool.tile([P, SL], F32, tag="r")
                        nc.tensor.matmul(rps, p_swap_r, qt,
                                         start=True, stop=True)
                        t1 = rstage.tile([P, SL], F32R, tag="t1")
                        nc.vector.tensor_tensor(
                            t1, qt.bitcast(F32),
                            cosT[:, sl * SL:(sl + 1) * SL], AluOpType.mult)
                        t2 = rstage.tile([P, SL], F32, tag="t2")
                        nc.vector.tensor_tensor(
                            t2, rps, sinT[:, sl * SL:(sl + 1) * SL], AluOpType.mult)
                        nc.vector.tensor_tensor(
                            t1, t1.bitcast(F32), t2, AluOpType.add)
                        nc.sync.dma_start(qkT_d[pt, :, sl * SL:(sl + 1) * SL], t1)

        # wot outlives phases C/E; opened only after hidT is freed
        wot_pool = ctx.enter_context(tc.tile_pool(name="wot", bufs=1))

        # ---------------- phases C+D: attention + Wo^T ----------------
        with ExitStack() as c:
            natd = c.enter_context(tc.tile_pool(name="natd", bufs=2))
            tpsd = c.enter_context(tc.tile_pool(name="tpsumd", bufs=2, space="PSUM"))
            wot = wot_pool.tile([P, HT, OC], BF16)  # 32 KB/part
            for ot in range(OC // P):
                def sink_wo(ct, ps, ot=ot):
                    nc.scalar.copy(wot[:, ct:ct + 4, ot * P:(ot + 1) * P], ps)
                _load_transposed(nc, wo_d[ot * P:(ot + 1) * P, :], natd, tpsd, sink_wo)

            qk_pool = c.enter_context(tc.tile_pool(name="qkio", bufs=2))
            v_pool = c.enter_context(tc.tile_pool(name="vio", bufs=2))
            e_pool = c.enter_context(tc.tile_pool(name="epool", bufs=20))
            z_pool = c.enter_context(tc.tile_pool(name="zpool", bufs=3))
            att_pool = c.enter_context(tc.tile_pool(name="attst", bufs=3))
            st_ps_pool = c.enter_context(
                tc.tile_pool(name="stpsum", bufs=2, space="PSUM"))
            u_ps_pool = c.enter_context(
                tc.tile_pool(name="upsum", bufs=2, space="PSUM"))
            z_ps_pool = c.enter_context(
                tc.tile_pool(name="zpsum", bufs=2, space="PSUM"))

            for h in range(NH_LOC):
                qT = qk_pool.tile([P, S], F32R, tag="q")
                kT = qk_pool.tile([P, S], F32R, tag="k")
                nc.sync.dma_start(qT, qkT_d[h])
                nc.sync.dma_start(kT, qkT_d[NH_LOC + h])
                vt = v_pool.tile([P, ST, D], F32R, tag="v")
                nc.sync.dma_start(
                    vt,
                    v_d[:, h * D:(h + 1) * D].rearrange("(t p) d -> p t d", p=P),
                )

                for j in range(NSL):
                    nkt = 4 * j + 4  # causal: k-tiles 0 .. 4j+3
                    etiles = []
                    for i in range(nkt):
                        r = i - 4 * j
                        off = max(0, r) * P
                        et = e_pool.tile([P, SL], F32R, tag="e")
                        st_ps = st_ps_pool.tile([P, SL], F32, tag="st")
                        nc.tensor.matmul(
                            st_ps[:, off:],
                            kT[:, i * P:(i + 1) * P],
                            qT[:, j * SL + off:(j + 1) * SL],
                            start=True, stop=True,
                        )
                        nc.scalar.activation(
                            et[:, off:], st_ps[:, off:], EXPF, scale=NORM)
                        if r >= 0:
                            nc.vector.tensor_tensor(
                                et[:, off:off + P],
                                et.bitcast(F32)[:, off:off + P],
                                tri01, AluOpType.mult)
                        etiles.append(et)

                    # sum of exp over k: full-width tiles are i <= 4j; the
                    # diagonal tiles r=1..3 only contribute on [128r:].
                    esum = z_pool.tile([P, SL], F32R, tag="es")
                    nfull = 4 * j + 1
                    if nfull >= 2:
                        nc.vector.tensor_tensor(
                            esum, etiles[0].bitcast(F32), etiles[1].bitcast(F32),
                            AluOpType.add)
                        first = 2
                    else:
                        nc.vector.tensor_copy(esum, etiles[0])
                        first = 1
                    for i in range(first, nfull):
                        nc.vector.tensor_tensor(
                            esum, esum.bitcast(F32), etiles[i].bitcast(F32),
                            AluOpType.add)
                    for i in range(nfull, nkt):
                        off = (i - 4 * j) * P
                        nc.vector.tensor_tensor(
                            esum[:, off:], esum.bitcast(F32)[:, off:],
                            etiles[i].bitcast(F32)[:, off:], AluOpType.add)
                    zb_ps = z_ps_pool.tile([P, SL], F32, tag="zb")
                    nc.tensor.matmul(zb_ps, ones_r, esum,
                                     start=True, stop=True)
                    zr = z_pool.tile([P, SL], F32, tag="zr")
                    nc.vector.reciprocal(zr, zb_ps)

                    u_ps = u_ps_pool.tile([P, SL], F32, tag="u")
                    for i in range(nkt):
                        r = i - 4 * j
                        off = max(0, r) * P
                        nc.tensor.matmul(
                            u_ps[:, off:],
                            vt[:, i, :],
                            etiles[i][:, off:],
                            start=(i == 0), stop=(i == nkt - 1),
                        )
                    att = att_pool.tile([P, SL], BF16, tag="a")
                    nc.vector.tensor_tensor(att, u_ps, zr, AluOpType.mult)
                    nc.sync.dma_start(
                        attn_loc[h * D:(h + 1) * D, j * SL:(j + 1) * SL], att)

                # gather this head now; overlaps the next heads' compute
                nc.gpsimd.collective_compute(
                    "AllGather", AluOpType.bypass,
                    replica_groups=[list(range(N_CORES))],
                    ins=[attn_loc[h * D:(h + 1) * D, :].opt()],
                    outs=[attn_all[h][:].opt()],
                )

        # ---------------- phase E: o_proj ----------------
        with ExitStack() as e:
            a_pool = e.enter_context(tc.tile_pool(name="apool", bufs=3))
            o_ps_pool = e.enter_context(
                tc.tile_pool(name="opsum", bufs=1, space="PSUM"))
            o_stage = e.enter_context(tc.tile_pool(name="ostage", bufs=4))
            NOT = OC // P  # 4 o-tiles
            for sh in range(2):  # s halves of 1024
                ops = [o_ps_pool.tile([P, SL], F32, tag=f"o{ot}_{sl}",
                                      name=f"ops{ot}_{sl}")
                       for ot in range(NOT) for sl in range(2)]
                # head-major order: c-tiles needing the last gathers go last
                for ci, (hh, cc) in enumerate(
                        (hh, cc) for hh in range(NH_LOC)
                        for cc in range(N_CORES)):
                    ct = cc * NH_LOC + hh
                    at = a_pool.tile([P, 2 * SL], BF16, tag="a")
                    nc.sync.dma_start(
                        at,
                        attn_all[hh][cc * P:(cc + 1) * P,
                                     sh * 2 * SL:(sh + 1) * 2 * SL],
                    )
                    for ot in range(NOT):
                        for sl in range(2):
                            nc.tensor.matmul(
                                ops[ot * 2 + sl],
                                wot[:, ct, ot * P:(ot + 1) * P],
                                at[:, sl * SL:(sl + 1) * SL],
                                start=(ci == 0), stop=(ci == HT - 1),
                            )
                for ot in range(NOT):
                    for sl in range(2):
                        ob = o_stage.tile([P, SL], F32, tag="ob")
                        nc.scalar.copy(ob, ops[ot * 2 + sl])
                        nc.sync.dma_start(
                            out_d[ot * P:(ot + 1) * P,
                                  sh * 2 * SL + sl * SL:
                                  sh * 2 * SL + (sl + 1) * SL],
                            ob,
                        )

    nc.compile()
    return nc


def make_in_maps(hidden_states, position_ids, W_pack, W_o):
    hidden = np.ascontiguousarray(
        np.asarray(hidden_states, dtype=np.float32).reshape(S, H))
    W_pack = np.asarray(W_pack, dtype=np.float32)
    W_o = np.asarray(W_o, dtype=np.float32)
    pos = np.asarray(position_ids).reshape(S).astype(np.float64)

    inv_freq = 1.0 / (10000.0 ** (np.arange(0, D, 2, dtype=np.float64) / D))
    freqs = np.outer(pos, inv_freq)  # [S, D/2]
    emb = np.concatenate([freqs, freqs], axis=1)  # [S, D]
    cos_t = np.ascontiguousarray(np.cos(emb).T.astype(np.float32))  # [D, S]
    sin_t = np.ascontiguousarray(np.sin(emb).T.astype(np.float32))

    in_maps = []
    for c in range(N_CORES):
        rows = []
        for g in range(3):  # q, k, v blocks of W_pack
            lo = g * H + c * OC
            rows.append(W_pack[lo:lo + OC])
        wpk = np.ascontiguousarray(np.concatenate(rows, axis=0))  # [1536, H]
        wo = np.ascontiguousarray(W_o[c * OC:(c + 1) * OC])  # [512, H]
        in_maps.append({
            "hidden": hidden,
            "w_pack": wpk,
            "w_o": wo,
            "cos_t": cos_t,
            "sin_t": sin_t,
        })
    return in_maps


_NC_CACHE = None


def get_nc():
    global _NC_CACHE
    if _NC_CACHE is None:
        _NC_CACHE = build_nc()
    return _NC_CACHE


def run(inputs, trace=False):
    """Run on hardware; returns (output [1,S,H] f32, BassKernelResults)."""
    in_maps = make_in_maps(
        inputs["hidden_states"], inputs["position_ids"],
        inputs["W_pack"], inputs["W_o"])
    nc = get_nc()
    res = run_bass_kernel_spmd(nc, in_maps, list(range(N_CORES)), trace=trace)
    parts = [np.asarray(res.results[c]["out_t"]) for c in range(N_CORES)]
    out_t = np.concatenate(parts, axis=0)  # [H, S]
    out = np.ascontiguousarray(out_t.T)[None]  # [1, S, H]
    return out.astype(np.float32), res


def kernel(**inputs):
    out, _ = run(inputs, trace=False)
    return out


# revision 65
# speedup vs baseline: 1.0131x; 1.0131x over previous
"""Trainium2 Bass kernel for nn_Attention_78280073937702.

Dense transformer attention block (prefill, B=1, S=2048, H=4096, 32 heads,
head_dim=128, fp32) sharded tensor-parallel over heads across 8 NeuronCores
(4 heads per core).  Per core:

  1. transpose hidden -> h-major (PE transpose, cast bf16)
  2. QKV projection (bf16 matmuls, fp32 accum):
       Q^T,K^T produced head-dim-major [d, s]; V produced natural [s, d]
  3. RoPE applied to Q^T/K^T (signed half-swap via PE permutation matmul +
     DVE elementwise with host-provided cos/sin tables)
  4. causal attention per head in fp32r:
       S^T[k,q] tiles -> exp (ACT) -> triangular-mask diag blocks ->
       Z via DVE tree-sum + ones-matmul colsum-broadcast ->
       U^T = V^T E (PSUM accum) -> attn^T = U^T * (1/Z)  (bf16)
  5. AllGather attn^T (bf16) across the 8 cores
  6. o_proj: out^T[o-slice, s] = Wo_slice @ attn_full^T (bf16 matmuls)

Host side: shards W_pack/W_o by head, builds RoPE tables from position_ids,
concatenates the 8 per-core out^T slices and transposes to [1, S, H].
"""

import os
import sys
from contextlib import ExitStack

import numpy as np

for _p in ("/opt/trn_rl_repo", os.path.expanduser("~/.axon_site/_ro/trn_rl_repo")):
    if os.path.isdir(_p) and _p not in sys.path:
        sys.path.insert(0, _p)

import concourse.bacc as bacc  # noqa: E402
import concourse.bass as bass  # noqa: E402
import concourse.mybir as mybir  # noqa: E402
import concourse.tile as tile  # noqa: E402
from concourse.alu_op_type import AluOpType  # noqa: E402
from concourse.bass_utils import run_bass_kernel_spmd  # noqa: E402

F32 = mybir.dt.float32
F32R = mybir.dt.float32r
BF16 = mybir.dt.bfloat16
EXPF = mybir.ActivationFunctionType.Exp

N_CORES = 8
S = 2048
H = 4096
D = 128
N_HEADS = 32
NH_LOC = N_HEADS // N_CORES  # 4 heads per core
P = 128
HT = H // P  # 32 h-tiles
ST = S // P  # 16 s-tiles
SL = 512  # s-slice width for matmul free dim
NSL = S // SL  # 4
PQK = 2 * NH_LOC  # 8 p-tiles for q,k
VC = NH_LOC * D  # 512 local v columns
OC = H // N_CORES  # 512 output columns per core
NORM = 1.0 / float(np.sqrt(D))


def r32(ap):
    return ap.bitcast(F32R)


def _load_transposed(nc, dram_row_block, nat_pool, tps_pool, sink):
    """Load a [P, H] fp32 row-block of `dram_row_block`, cast to bf16 on DVE,
    and PE-transpose (bf16, 2x faster) into h-major tiles 4 at a time (one
    PSUM bank), delivering each [P, 4, P] bf16 PSUM tile to
    sink(ht_base, psum_tile)."""
    for half in range(2):
        nt = nat_pool.tile([P, H // 2], F32, tag="nat")
        nc.sync.dma_start(nt, dram_row_block[:, half * (H // 2):(half + 1) * (H // 2)])
        ntb = nat_pool.tile([P, H // 2], BF16, tag="natb")
        nc.vector.tensor_copy(ntb, nt)
        for grp in range(HT // 8):
            ps = tps_pool.tile([P, 4, P], BF16, tag="t")
            for u in range(4):
                hh = grp * 4 + u
                nc.tensor.transpose(
                    ps[:, u, :], ntb[:, hh * P:(hh + 1) * P], _load_transposed.identb)
            sink(half * (HT // 2) + grp * 4, ps)


def build_nc():
    nc = bacc.Bacc("TRN2", target_bir_lowering=False, num_devices=N_CORES)

    hid_d = nc.dram_tensor("hidden", [S, H], F32, kind="ExternalInput")
    wpk_d = nc.dram_tensor("w_pack", [PQK * P + VC, H], F32, kind="ExternalInput")
    wo_d = nc.dram_tensor("w_o", [OC, H], F32, kind="ExternalInput")
    cos_d = nc.dram_tensor("cos_t", [D, S], F32, kind="ExternalInput")
    sin_d = nc.dram_tensor("sin_t", [D, S], F32, kind="ExternalInput")
    out_d = nc.dram_tensor("out_t", [OC, S], F32, kind="ExternalOutput")

    with tile.TileContext(nc) as tc, ExitStack() as ctx:
        dram = ctx.enter_context(tc.tile_pool(name="dram", bufs=1, space="DRAM"))
        qkT_d = dram.tile([PQK, P, S], F32R)  # roped q^T,k^T per head
        v_d = dram.tile([S, VC], F32R)  # V natural, all local heads
        attn_loc = dram.tile([OC, S], BF16)
        # one gather buffer per local head index: rows = (core, d)
        attn_all = [
            dram.tile([N_CORES * D, S], BF16, addr_space="Shared",
                      name=f"attn_all{h}")
            for h in range(NH_LOC)
        ]

        # ---------------- constants ----------------
        consts = ctx.enter_context(tc.tile_pool(name="consts", bufs=1))
        ones_t = consts.tile([P, P], F32)
        nc.gpsimd.memset(ones_t, 1.0)
        ident = consts.tile([P, P], F32)
        nc.gpsimd.memset(ident, 0.0)
        nc.gpsimd.affine_select(
            out=ident, in_=ones_t, compare_op=AluOpType.is_equal,
            fill=0.0, base=0, channel_multiplier=1, pattern=[[-1, P]],
        )
        identb = consts.tile([P, P], BF16)
        nc.vector.tensor_copy(identb, ident)
        _load_transposed.identb = identb
        # signed half-swap as lhsT (so that lhsT.T @ x = rot_half(x)):
        # lhsT[i, i+64] = +1 (i < 64), lhsT[i, i-64] = -1 (i >= 64)
        neg_t = consts.tile([P, P], F32)
        nc.gpsimd.memset(neg_t, -1.0)
        sw_pos = consts.tile([P, P], F32)
        nc.gpsimd.affine_select(
            out=sw_pos, in_=ones_t, compare_op=AluOpType.is_equal,
            fill=0.0, base=-64, channel_multiplier=-1, pattern=[[1, P]],
        )  # iota = -64 - i + j == 0  ->  j = i + 64
        sw_neg = consts.tile([P, P], F32)
        nc.gpsimd.affine_select(
            out=sw_neg, in_=neg_t, compare_op=AluOpType.is_equal,
            fill=0.0, base=64, channel_multiplier=-1, pattern=[[1, P]],
        )  # iota = 64 - i + j == 0  ->  j = i - 64
        p_swap = consts.tile([P, P], F32)
        nc.vector.tensor_add(p_swap, sw_pos, sw_neg)
        # upper-triangular-with-diag 0/1 keep-mask in [k, q] layout:
        # keep where q - k >= 0
        tri01 = consts.tile([P, P], F32)
        nc.gpsimd.affine_select(
            out=tri01, in_=ones_t, compare_op=AluOpType.is_ge,
            fill=0.0, base=0, channel_multiplier=-1, pattern=[[1, P]],
        )
        # fp32r copies for fp32r matmuls (producer must round to f32r)
        p_swap_r = consts.tile([P, P], F32R)
        nc.vector.tensor_copy(p_swap_r, p_swap)
        ones_r = consts.tile([P, P], F32R)
        nc.vector.tensor_copy(ones_r, ones_t)

        with ExitStack() as ab:  # phases A, B1, B2: hidT + loaders live here
            nat = ab.enter_context(tc.tile_pool(name="nat", bufs=2))
            tps = ab.enter_context(tc.tile_pool(name="tpsum", bufs=2, space="PSUM"))
            hidT_pool = ab.enter_context(tc.tile_pool(name="hidT", bufs=1))
            hidT = hidT_pool.tile([P, HT, S], BF16)  # 128 KB/part

            # ---------- phases A+B1 interleaved: hidden^T + V projection ----
            # wtv is built first so the V matmuls can cover the PE while the
            # (DMA-bound) hidden loads stream in.
            with ExitStack() as b1:
                wtv_pool = b1.enter_context(tc.tile_pool(name="wtv", bufs=1))
                vps_pool = b1.enter_context(
                    tc.tile_pool(name="vpsum", bufs=2, space="PSUM"))
                vstage = b1.enter_context(tc.tile_pool(name="vstage", bufs=2))
                wtv = wtv_pool.tile([P, HT, VC], BF16)  # 32 KB/part
                for i in range(NH_LOC):
                    pt = PQK + i
                    def sink_w(ht, ps, i=i):
                        nc.scalar.copy(wtv[:, ht:ht + 4, i * P:(i + 1) * P], ps)
                    _load_transposed(
                        nc, wpk_d[pt * P:(pt + 1) * P, :], nat, tps, sink_w)
                for st in range(ST):
                    def sink_hid(ht, ps, st=st):
                        nc.scalar.copy(hidT[:, ht:ht + 4, st * P:(st + 1) * P], ps)
                    _load_transposed(
                        nc, hid_d[st * P:(st + 1) * P, :], nat, tps, sink_hid)
                    vps = vps_pool.tile([P, VC], F32, tag="v")
                    for ht in range(HT):
                        nc.tensor.matmul(
                            vps, hidT[:, ht, st * P:(st + 1) * P], wtv[:, ht, :],
                            start=(ht == 0), stop=(ht == HT - 1),
                        )
                    vsb = vstage.tile([P, VC], F32R, tag="vs")
                    nc.scalar.copy(vsb, vps)
                    nc.sync.dma_start(v_d[st * P:(st + 1) * P, :], vsb)

            # ---------------- phase B2: Q/K projection + RoPE ----------------
            with ExitStack() as b2:
                trig = b2.enter_context(tc.tile_pool(name="trig", bufs=1))
                cosT = trig.tile([D, S], F32)
                sinT = trig.tile([D, S], F32)
                nc.sync.dma_start(cosT, cos_d[:, :])
                nc.sync.dma_start(sinT, sin_d[:, :])

                wt_pool = b2.enter_context(tc.tile_pool(name="wt", bufs=1))
                qkps_pool = b2.enter_context(
                    tc.tile_pool(name="qkpsum", bufs=1, space="PSUM"))
                rps_pool = b2.enter_context(
                    tc.tile_pool(name="ropepsum", bufs=2, space="PSUM"))
                rstage = b2.enter_context(tc.tile_pool(name="rstage", bufs=2))

                for pt in range(PQK):
                    wt = wt_pool.tile([P, HT, P], BF16, tag="wt")
                    def sink_qk(ht, ps, wt=wt):
                        nc.scalar.copy(wt[:, ht:ht + 4, :], ps)
                    _load_transposed(
                        nc, wpk_d[pt * P:(pt + 1) * P, :], nat, tps, sink_qk)

                    qk_ps = [qkps_pool.tile([P, SL], F32, tag=f"qk{sl}",
                                            name=f"qkps{sl}")
                             for sl in range(NSL)]
                    for ht in range(HT):
                        for sl in range(NSL):
                            nc.tensor.matmul(
                                qk_ps[sl], wt[:, ht, :],
                                hidT[:, ht, sl * SL:(sl + 1) * SL],
                                start=(ht == 0), stop=(ht == HT - 1),
                            )
                    for sl in range(NSL):
                        qt = rstage.tile([P, SL], F32R, tag="qt")
                        nc.scalar.copy(qt, qk_ps[sl])
                        rps = rps_pool.tile([P, SL], F32, tag="r")
                        nc.tensor.matmul(rps, p_swap_r, qt,
                                         start=True, stop=True)
                        t1 = rstage.tile([P, SL], F32R, tag="t1")
                        nc.vector.tensor_tensor(
                            t1, qt.bitcast(F32),
                            cosT[:, sl * SL:(sl + 1) * SL], AluOpType.mult)
                        t2 = rstage.tile([P, SL], F32, tag="t2")
                        nc.vector.tensor_tensor(
                            t2, rps, sinT[:, sl * SL:(sl + 1) * SL], AluOpType.mult)
                        nc.vector.tensor_tensor(
                            t1, t1.bitcast(F32), t2, AluOpType.add)
                        nc.sync.dma_start(qkT_d[pt, :, sl * SL:(sl + 1) * SL], t1)

        # wot outlives phases C/E; opened only after hidT is freed
        wot_pool = ctx.enter_context(tc.tile_pool(name="wot", bufs=1))

        # ---------------- phases C+D: attention + Wo^T ----------------
        with ExitStack() as c:
            natd = c.enter_context(tc.tile_pool(name="natd", bufs=2))
            tpsd = c.enter_context(tc.tile_pool(name="tpsumd", bufs=2, space="PSUM"))
            wot = wot_pool.tile([P, HT, OC], BF16)  # 32 KB/part
            for ot in range(OC // P):
                def sink_wo(ct, ps, ot=ot):
                    nc.scalar.copy(wot[:, ct:ct + 4, ot * P:(ot + 1) * P], ps)
                _load_transposed(nc, wo_d[ot * P:(ot + 1) * P, :], natd, tpsd, sink_wo)

            qk_pool = c.enter_context(tc.tile_pool(name="qkio", bufs=2))
            v_pool = c.enter_context(tc.tile_pool(name="vio", bufs=2))
            e_pool = c.enter_context(tc.tile_pool(name="epool", bufs=20))
            z_pool = c.enter_context(tc.tile_pool(name="zpool", bufs=3))
            att_pool = c.enter_context(tc.tile_pool(name="attst", bufs=3))
            st_ps_pool = c.enter_context(
                tc.tile_pool(name="stpsum", bufs=2, space="PSUM"))
            u_ps_pool = c.enter_context(
                tc.tile_pool(name="upsum", bufs=2, space="PSUM"))
            z_ps_pool = c.enter_context(
                tc.tile_pool(name="zpsum", bufs=2, space="PSUM"))

            for h in range(NH_LOC):
                qT = qk_pool.tile([P, S], F32R, tag="q")
                kT = qk_pool.tile([P, S], F32R, tag="k")
                nc.sync.dma_start(qT, qkT_d[h])
                nc.sync.dma_start(kT, qkT_d[NH_LOC + h])
                vt = v_pool.tile([P, ST, D], F32R, tag="v")
                nc.sync.dma_start(
                    vt,
                    v_d[:, h * D:(h + 1) * D].rearrange("(t p) d -> p t d", p=P),
                )

                for j in range(NSL):
                    nkt = 4 * j + 4  # causal: k-tiles 0 .. 4j+3
                    etiles = []
                    for i in range(nkt):
                        r = i - 4 * j
                        off = max(0, r) * P
                        et = e_pool.tile([P, SL], F32R, tag="e")
                        st_ps = st_ps_pool.tile([P, SL], F32, tag="st")
                        nc.tensor.matmul(
                            st_ps[:, off:],
                            kT[:, i * P:(i + 1) * P],
                            qT[:, j * SL + off:(j + 1) * SL],
                            start=True, stop=True,
                        )
                        nc.scalar.activation(
                            et[:, off:], st_ps[:, off:], EXPF, scale=NORM)
                        if r >= 0:
                            nc.vector.tensor_tensor(
                                et[:, off:off + P],
                                et.bitcast(F32)[:, off:off + P],
                                tri01, AluOpType.mult)
                        etiles.append(et)

                    # sum of exp over k: full-width tiles are i <= 4j; the
                    # diagonal tiles r=1..3 only contribute on [128r:].
                    esum = z_pool.tile([P, SL], F32R, tag="es")
                    nfull = 4 * j + 1
                    if nfull >= 2:
                        nc.vector.tensor_tensor(
                            esum, etiles[0].bitcast(F32), etiles[1].bitcast(F32),
                            AluOpType.add)
                        first = 2
                    else:
                        nc.vector.tensor_copy(esum, etiles[0])
                        first = 1
                    for i in range(first, nfull):
                        nc.vector.tensor_tensor(
                            esum, esum.bitcast(F32), etiles[i].bitcast(F32),
                            AluOpType.add)
                    for i in range(nfull, nkt):
                        off = (i - 4 * j) * P
                        nc.vector.tensor_tensor(
                            esum[:, off:], esum.bitcast(F32)[:, off:],
                            etiles[i].bitcast(F32)[:, off:], AluOpType.add)
                    zb_ps = z_ps_pool.tile([P, SL], F32, tag="zb")
                    nc.tensor.matmul(zb_ps, ones_r, esum,
                                     start=True, stop=True)
                    zr = z_pool.tile([P, SL], F32, tag="zr")
                    nc.vector.reciprocal(zr, zb_ps)

                    u_ps = u_ps_pool.tile([P, SL], F32, tag="u")
                    for i in range(nkt):
                        r = i - 4 * j
                        off = max(0, r) * P
                        nc.tensor.matmul(
                            u_ps[:, off:],
                            vt[:, i, :],
                            etiles[i][:, off:],
                            start=(i == 0), stop=(i == nkt - 1),
                        )
                    att = att_pool.tile([P, SL], BF16, tag="a")
                    nc.vector.tensor_tensor(att, u_ps, zr, AluOpType.mult)
                    nc.sync.dma_start(
                        attn_loc[h * D:(h + 1) * D, j * SL:(j + 1) * SL], att)

                # gather this head now; overlaps the next heads' compute
                nc.gpsimd.collective_compute(
                    "AllGather", AluOpType.bypass,
                    replica_groups=[list(range(N_CORES))],
                    ins=[attn_loc[h * D:(h + 1) * D, :].opt()],
                    outs=[attn_all[h][:].opt()],
                )

        # ---------------- phase E: o_proj ----------------
        with ExitStack() as e:
            a_pool = e.enter_context(tc.tile_pool(name="apool", bufs=3))
            o_ps_pool = e.enter_context(
                tc.tile_pool(name="opsum", bufs=1, space="PSUM"))
            o_stage = e.enter_context(tc.tile_pool(name="ostage", bufs=4))
            NOT = OC // P  # 4 o-tiles
            for sh in range(2):  # s halves of 1024
                ops = [o_ps_pool.tile([P, SL], F32, tag=f"o{ot}_{sl}",
                                      name=f"ops{ot}_{sl}")
                       for ot in range(NOT) for sl in range(2)]
                # head-major order: c-tiles needing the last gathers go last
                for ci, (hh, cc) in enumerate(
                        (hh, cc) for hh in range(NH_LOC)
                        for cc in range(N_CORES)):
                    ct = cc * NH_LOC + hh
                    at = a_pool.tile([P, 2 * SL], BF16, tag="a")
                    nc.sync.dma_start(
                        at,
                        attn_all[hh][cc * P:(cc + 1) * P,
                                     sh * 2 * SL:(sh + 1) * 2 * SL],
                    )
                    for ot in range(NOT):
                        for sl in range(2):
                            nc.tensor.matmul(
                                ops[ot * 2 + sl],
                                wot[:, ct, ot * P:(ot + 1) * P],
                                at[:, sl * SL:(sl + 1) * SL],
                                start=(ci == 0), stop=(ci == HT - 1),
                            )
                for ot in range(NOT):
                    for sl in range(2):
                        ob = o_stage.tile([P, SL], F32, tag="ob")
                        nc.scalar.copy(ob, ops[ot * 2 + sl])
                        nc.sync.dma_start(
                            out_d[ot * P:(ot + 1) * P,
                                  sh * 2 * SL + sl * SL:
                                  sh * 2 * SL + (sl + 1) * SL],
                            ob,
                        )

    nc.compile()
    return nc


def make_in_maps(hidden_states, position_ids, W_pack, W_o):
    hidden = np.ascontiguousarray(
        np.asarray(hidden_states, dtype=np.float32).reshape(S, H))
    W_pack = np.asarray(W_pack, dtype=np.float32)
    W_o = np.asarray(W_o, dtype=np.float32)
    pos = np.asarray(position_ids).reshape(S).astype(np.float64)

    inv_freq = 1.0 / (10000.0 ** (np.arange(0, D, 2, dtype=np.float64) / D))
    freqs = np.outer(pos, inv_freq)  # [S, D/2]
    emb = np.concatenate([freqs, freqs], axis=1)  # [S, D]
    cos_t = np.ascontiguousarray(np.cos(emb).T.astype(np.float32))  # [D, S]
    sin_t = np.ascontiguousarray(np.sin(emb).T.astype(np.float32))

    in_maps = []
    for c in range(N_CORES):
        rows = []
        for g in range(3):  # q, k, v blocks of W_pack
            lo = g * H + c * OC
            rows.append(W_pack[lo:lo + OC])
        wpk = np.ascontiguousarray(np.concatenate(rows, axis=0))  # [1536, H]
        wo = np.ascontiguousarray(W_o[c * OC:(c + 1) * OC])  # [512, H]
        in_maps.append({
            "hidden": hidden,
            "w_pack": wpk,
            "w_o": wo,
            "cos_t": cos_t,
            "sin_t": sin_t,
        })
    return in_maps


_NC_CACHE = None


def get_nc():
    global _NC_CACHE
    if _NC_CACHE is None:
        _NC_CACHE = build_nc()
    return _NC_CACHE


def run(inputs, trace=False):
    """Run on hardware; returns (output [1,S,H] f32, BassKernelResults)."""
    in_maps = make_in_maps(
        inputs["hidden_states"], inputs["position_ids"],
        inputs["W_pack"], inputs["W_o"])
    nc = get_nc()
    res = run_bass_kernel_spmd(nc, in_maps, list(range(N_CORES)), trace=trace)
    parts = [np.asarray(res.results[c]["out_t"]) for c in range(N_CORES)]
    out_t = np.concatenate(parts, axis=0)  # [H, S]
    out = np.ascontiguousarray(out_t.T)[None]  # [1, S, H]
    return out.astype(np.float32), res


def kernel(**inputs):
    out, _ = run(inputs, trace=False)
    return out
